# revision 12
# baseline (speedup 1.0000x reference)
"""Bass/Tile TRN2 kernel for per-token multi-head attention over heads.

Reference computation (per token t):
  qkv = x @ w_qkv + b_qkv                  # [t, 3072]
  q/k/v[h, d] = qkv[h*192 + {0,64,128} + d]
  scores[h, g] = q[h] . k[g] / 8
  attn = softmax(scores, axis=g)
  out[h, d] = sum_g attn[h, g] v[g, d]
  y = out.reshape(1024) @ w_out + b_out

Sharding: tokens (B*S = 32768) split evenly over 8 cores; weights replicated.

Layout notes (v2):
  - qkv computed transposed ([f x t]) so per-head 64-row slabs are clean
    partition ranges; f32r matmuls (full PE rate at N>=256, ~1e-4 rounding).
  - per-head q/k/v repacked h-major into [d, head, t] bf16 tiles with
    contiguous copies; block-diag 128x128 matmuls give 8 tokens' 16x16
    head-attention at once (2 groups share each psum tile). Mask selects
    t==t' pairs; exp is unnormalized, softmax denominator comes from a
    ones-column in the AV matmul.
  - x transposed via DMA-transpose (2 x 64-partition transfers per e-chunk).
"""

import numpy as np
import ml_dtypes

H, DH = 16, 64
E = 1024
F3 = 3072
B, S = 4, 8192
N_CORES = 8
TOKS = (B * S) // N_CORES  # 4096 tokens per core
T = 256                    # tokens per unrolled iteration
NG = T // 8                # 8-token groups per iteration

NEG = -1.0e9


def _consts():
    # scoresT rows a=(t, g) t-major, cols b=(h, t') h-major; valid iff t==t'
    a = np.arange(128)
    mask = np.where((a[:, None] // 16) == (a[None, :] % 8), 0.0, NEG).astype(
        np.float32
    )
    mask2 = np.concatenate([mask, mask], axis=1)  # [128, 256] for group pairs
    ident = np.eye(128, dtype=np.float32)
    return mask2, ident


def build(toks_per_core=TOKS):
    from concourse.bacc import Bacc
    import concourse.mybir as mybir
    from concourse.tile import TileContext
    from concourse.bass import ds

    f32 = mybir.dt.float32
    f32r = mybir.dt.float32r
    bf16 = mybir.dt.bfloat16
    niter = toks_per_core // T

    nc = Bacc("TRN2")
    x_d = nc.dram_tensor("x", [toks_per_core, E], f32r, kind="ExternalInput")
    wqkv_d = nc.dram_tensor("w_qkv", [E, F3], f32r, kind="ExternalInput")
    bqkv_d = nc.dram_tensor("b_qkv", [128, F3 // 128], f32, kind="ExternalInput")
    wout_d = nc.dram_tensor("w_out", [E, E], bf16, kind="ExternalInput")
    bout_d = nc.dram_tensor("b_out", [1, E], f32r, kind="ExternalInput")
    out_d = nc.dram_tensor("out", [toks_per_core, E], f32, kind="ExternalOutput")

    mask2_np, ident_np = _consts()
    mask2_c = nc.inline_tensor(mask2_np, name="mask2_c")
    identb_c = nc.inline_tensor(ident_np.astype(ml_dtypes.bfloat16), name="identb_c")
    ones_c = nc.inline_tensor(np.ones((1, 128), np.float32), name="ones_c")

    with TileContext(nc) as tc:
        with (
            tc.tile_pool(name="persist", bufs=1) as pp,
            tc.tile_pool(name="xtp", bufs=2) as xtp,
            tc.tile_pool(name="qkvtp", bufs=4) as qkvtp,
            tc.tile_pool(name="attnsb", bufs=4) as attnsb,
            tc.tile_pool(name="stagep", bufs=1) as stagep,
            tc.tile_pool(name="outtokp", bufs=2) as outtokp,
            tc.tile_pool(name="outfp", bufs=3) as outfp,
            tc.tile_pool(name="psbig", bufs=4, space="PSUM") as psbig,
            tc.tile_pool(name="psattn", bufs=4, space="PSUM") as psattn,
        ):
            # ---- resident weights / constants ----
            w_sb = pp.tile([128, 8, F3], f32r)
            nc.sync.dma_start(w_sb, wqkv_d.rearrange("(ko kp) f -> kp ko f", kp=128))
            wout_sb = pp.tile([128, 8, E], bf16)
            nc.sync.dma_start(wout_sb, wout_d.rearrange("(ko kp) f -> kp ko f", kp=128))
            bqkv_sb = pp.tile([128, F3 // 128], f32)
            nc.sync.dma_start(bqkv_sb, bqkv_d[:])
            bout_sb = pp.tile([1, E], f32r)
            nc.sync.dma_start(bout_sb, bout_d[:])
            mask2_sb = pp.tile([128, 256], f32)
            nc.sync.dma_start(mask2_sb, mask2_c[:])
            idb_sb = pp.tile([128, 128], bf16)
            nc.sync.dma_start(idb_sb, identb_c[:])
            ones_sb = pp.tile([1, 128], f32r)
            nc.sync.dma_start(ones_sb, ones_c[:].bitcast(f32r))

            # persistent packs: q h-major [d, head, t] (moving operand),
            # k/v t-major [d, t, head] (stationary needs one free dim)
            qpack = pp.tile([64, H, T], bf16)
            kpack = pp.tile([64, T, H], bf16)
            vpack = pp.tile([65, T, H], bf16)
            nc.gpsimd.memset(vpack[64:65, :, :], 1.0)

            for it in range(niter):
                t0 = it * T
                # ---- xT [e, t] via DMA transpose (64-partition halves) ----
                xt_sb = xtp.tile([128, 8, T], f32r)
                for e in range(8):
                    for hp in range(2):
                        nc.sync.dma_start_transpose(
                            xt_sb[hp * 64 : hp * 64 + 64, e, :],
                            x_d[ds(t0, T), ds(e * 128 + hp * 64, 64)],
                        )

                # ---- QKV projection: qkvT tiles [128f x T] ----
                qkvt = []
                for j in range(F3 // 128):
                    psq_full = psbig.tile([128, 512], f32, tag="ps_big")
                    psq = psq_full[:, :T]
                    for e in range(8):
                        nc.tensor.matmul(
                            psq,
                            w_sb[:, e, ds(j * 128, 128)],
                            xt_sb[:, e, :],
                            start=(e == 0),
                            stop=(e == 7),
                        )
                    qt = qkvtp.tile([128, T], bf16, tag="qkvt")
                    nc.scalar.activation(
                        qt[:],
                        psq,
                        mybir.ActivationFunctionType.Identity,
                        bias=bqkv_sb[:, j : j + 1],
                        scale=1.0,
                    )
                    qkvt.append(qt)

                def slab(frow):
                    jt, off = divmod(frow, 128)
                    return qkvt[jt][off : off + 64, :]

                for h in range(H):
                    nc.vector.tensor_copy(out=qpack[:, h, :], in_=slab(192 * h))
                    nc.vector.tensor_copy(
                        out=kpack[:, :, h], in_=slab(192 * h + 64)
                    )
                    nc.scalar.copy(
                        out=vpack[:64, :, h], in_=slab(192 * h + 128)
                    )

                # ---- attention: 8-token groups, processed in pairs ----
                staging = stagep.tile([64, T, H], bf16, tag="staging")
                for gp in range(NG // 2):
                    gA, gB = 2 * gp, 2 * gp + 1
                    # scoresT for the pair -> one psum tile
                    psS2 = psattn.tile([128, 2, 128], f32, tag="ps_attn")
                    for i, g in enumerate((gA, gB)):
                        nc.tensor.matmul(
                            psS2[:, i, :],
                            kpack[:, ds(g * 8, 8), :].rearrange(
                                "p a b -> p (a b)"
                            ),
                            qpack[:, :, ds(g * 8, 8)],
                            start=True,
                            stop=True,
                        )
                    smask2 = attnsb.tile([128, 256], f32, tag="smask")
                    nc.vector.tensor_add(
                        out=smask2[:],
                        in0=psS2.rearrange("p a b -> p (a b)"),
                        in1=mask2_sb[:],
                    )
                    expS2 = attnsb.tile([128, 256], bf16, tag="expS")
                    nc.scalar.activation(
                        expS2[:],
                        smask2[:],
                        mybir.ActivationFunctionType.Exp,
                        bias=0.0,
                        scale=0.125,
                    )
                    # vT for the pair: [(g,t) rows] x [d|1 cols]
                    psV2 = psattn.tile([128, 2, 66], bf16, tag="ps_attn")
                    for i, g in enumerate((gA, gB)):
                        nc.tensor.transpose(
                            psV2[:, i, :65],
                            vpack[:, ds(g * 8, 8), :].rearrange(
                                "p a b -> p (a b)"
                            ),
                            idb_sb[:65, :65],
                        )
                    vt2_sb = attnsb.tile([128, 2, 65], bf16, tag="vt")
                    nc.scalar.activation(
                        vt2_sb[:],
                        psV2[:, :, :65],
                        mybir.ActivationFunctionType.Copy,
                    )
                    # AV (+ denominator in col 64)
                    psAV2 = psattn.tile([128, 2, 65], f32, tag="ps_attn")
                    for i in range(2):
                        nc.tensor.matmul(
                            psAV2[:, i, :],
                            expS2[:, ds(i * 128, 128)],
                            vt2_sb[:, i, :],
                            start=True,
                            stop=True,
                        )
                    rec2 = attnsb.tile([128, 2], f32, tag="rec")
                    nc.vector.reciprocal(rec2[:], psAV2[:, :, 64])
                    onorm2 = attnsb.tile([128, 2, 64], bf16, tag="onorm")
                    nc.vector.tensor_tensor(
                        onorm2[:],
                        psAV2[:, :, 0:64],
                        rec2[:, :, None].to_broadcast((128, 2, 64)),
                        mybir.AluOpType.mult,
                    )
                    # transpose [(h,t), d-pair] -> [d-pair, (h,t)]
                    psN2 = psattn.tile([128, 128], bf16, tag="ps_attn")
                    nc.tensor.transpose(
                        psN2,
                        onorm2.rearrange("p a b -> p (a b)"),
                        idb_sb[:],
                    )
                    for i, g in enumerate((gA, gB)):
                        nc.vector.tensor_copy(
                            out=staging[:, ds(g * 8, 8), :],
                            in_=psN2[i * 64 : i * 64 + 64, :]
                            .rearrange("p (a b) -> p a b", a=H)
                            .rearrange("p a b -> p b a"),
                        )

                # ---- regroup to outtok [(h*64+d) chunks, t] ----
                outtok = outtokp.tile([128, 8, T], bf16, tag="outtok")
                for h in range(H):
                    nc.gpsimd.tensor_copy(
                        out=outtok[(h % 2) * 64 : (h % 2) * 64 + 64, h // 2, :],
                        in_=staging[:, :, h],
                    )

                # ---- output projection + bias ----
                for jm in range(T // 128):
                    for nh in range(2):
                        psO = psbig.tile([128, 512], f32, tag="ps_big")
                        for k2 in range(8):
                            nc.tensor.matmul(
                                psO,
                                outtok[:, k2, ds(jm * 128, 128)],
                                wout_sb[:, k2, ds(nh * 512, 512)],
                                start=(k2 == 0),
                                stop=False,
                            )
                        nc.tensor.matmul(
                            psO,
                            ones_sb[:, :],
                            bout_sb[:, ds(nh * 512, 512)],
                            start=False,
                            stop=True,
                        )
                        outf = outfp.tile([128, 512], f32, tag="outf")
                        nc.scalar.activation(
                            outf[:], psO, mybir.ActivationFunctionType.Copy
                        )
                        nc.sync.dma_start(
                            out_d[ds(t0 + jm * 128, 128), ds(nh * 512, 512)],
                            outf[:],
                        )
    nc.finalize()
    return nc


_cache = {}


def _get_nc(toks_per_core=TOKS):
    if toks_per_core not in _cache:
        _cache[toks_per_core] = build(toks_per_core)
    return _cache[toks_per_core]


def prep_inputs(x, w_qkv, b_qkv, w_out, b_out, toks_per_core=TOKS, n_cores=N_CORES):
    """Shard tokens over cores; replicate (host-preprocessed) weights."""
    xf = np.ascontiguousarray(x, dtype=np.float32).reshape(-1, E)
    wq = np.ascontiguousarray(w_qkv, dtype=np.float32)
    bq = np.ascontiguousarray(
        np.asarray(b_qkv, dtype=np.float32).reshape(F3 // 128, 128).T
    )
    wo = np.ascontiguousarray(np.asarray(w_out).astype(ml_dtypes.bfloat16))
    bo = np.ascontiguousarray(np.asarray(b_out, dtype=np.float32).reshape(1, E))
    in_maps = []
    for c in range(n_cores):
        in_maps.append(
            {
                "x": np.ascontiguousarray(
                    xf[c * toks_per_core : (c + 1) * toks_per_core]
                ),
                "w_qkv": wq,
                "b_qkv": bq,
                "w_out": wo,
                "b_out": bo,
            }
        )
    return in_maps


def run(x, w_qkv, b_qkv, w_out, b_out, toks_per_core=TOKS, n_cores=N_CORES, **kw):
    from concourse import bass_utils

    nc = _get_nc(toks_per_core)
    in_maps = prep_inputs(
        x, w_qkv, b_qkv, w_out, b_out, toks_per_core, n_cores
    )
    res = bass_utils.run_bass_kernel_spmd(
        nc, in_maps, core_ids=list(range(n_cores)), **kw
    )
    out = np.concatenate([r["out"] for r in res.results], axis=0)
    return out, res


def kernel(x, w_qkv, b_qkv, w_out, b_out):
    out, _ = run(x, w_qkv, b_qkv, w_out, b_out)
    return out.reshape(x.shape[0], x.shape[1], E)


# revision 13
# speedup vs baseline: 2.0638x; 2.0638x over previous
"""Bass/Tile TRN2 kernel for per-token multi-head attention over heads.

Reference computation (per token t):
  qkv = x @ w_qkv + b_qkv                  # [t, 3072]
  q/k/v[h, d] = qkv[h*192 + {0,64,128} + d]
  scores[h, g] = q[h] . k[g] / 8
  attn = softmax(scores, axis=g)
  out[h, d] = sum_g attn[h, g] v[g, d]
  y = out.reshape(1024) @ w_out + b_out

Sharding: tokens (B*S = 32768) split evenly over 8 cores; weights replicated.

Layout notes (v2):
  - qkv computed transposed ([f x t]) so per-head 64-row slabs are clean
    partition ranges; f32r matmuls (full PE rate at N>=256, ~1e-4 rounding).
  - per-head q/k/v repacked h-major into [d, head, t] bf16 tiles with
    contiguous copies; block-diag 128x128 matmuls give 8 tokens' 16x16
    head-attention at once (2 groups share each psum tile). Mask selects
    t==t' pairs; exp is unnormalized, softmax denominator comes from a
    ones-column in the AV matmul.
  - x transposed via DMA-transpose (2 x 64-partition transfers per e-chunk).
"""

import numpy as np
import ml_dtypes

H, DH = 16, 64
E = 1024
F3 = 3072
B, S = 4, 8192
N_CORES = 8
TOKS = (B * S) // N_CORES  # 4096 tokens per core
T = 256                    # tokens per unrolled iteration
NG = T // 8                # 8-token groups per iteration

NEG = -1.0e9


def _consts():
    # scoresT rows a=(t, g) t-major, cols b=(h, t') h-major; valid iff t==t'
    a = np.arange(128)
    mask = np.where((a[:, None] // 16) == (a[None, :] % 8), 0.0, NEG).astype(
        np.float32
    )
    mask2 = np.concatenate([mask, mask], axis=1)  # [128, 256] for group pairs
    ident = np.eye(128, dtype=np.float32)
    return mask2, ident


def build(toks_per_core=TOKS):
    from concourse.bacc import Bacc
    import concourse.mybir as mybir
    from concourse.tile import TileContext
    from concourse.bass import ds

    f32 = mybir.dt.float32
    f32r = mybir.dt.float32r
    bf16 = mybir.dt.bfloat16
    niter = toks_per_core // T

    nc = Bacc("TRN2")
    x_d = nc.dram_tensor("x", [toks_per_core, E], f32r, kind="ExternalInput")
    wqkv_d = nc.dram_tensor("w_qkv", [E, F3], f32r, kind="ExternalInput")
    bqkv_d = nc.dram_tensor("b_qkv", [128, F3 // 128], f32, kind="ExternalInput")
    wout_d = nc.dram_tensor("w_out", [E, E], bf16, kind="ExternalInput")
    bout_d = nc.dram_tensor("b_out", [1, E], f32r, kind="ExternalInput")
    out_d = nc.dram_tensor("out", [toks_per_core, E], f32, kind="ExternalOutput")

    mask2_np, ident_np = _consts()
    mask2_c = nc.inline_tensor(mask2_np, name="mask2_c")
    identf_c = nc.inline_tensor(ident_np, name="identf_c")
    identb_c = nc.inline_tensor(ident_np.astype(ml_dtypes.bfloat16), name="identb_c")
    ones_c = nc.inline_tensor(np.ones((1, 128), np.float32), name="ones_c")

    with TileContext(nc) as tc:
        with (
            tc.tile_pool(name="persist", bufs=1) as pp,
            tc.tile_pool(name="xtp", bufs=2) as xtp,
            tc.tile_pool(name="qkvtp", bufs=4) as qkvtp,
            tc.tile_pool(name="attnsb", bufs=4) as attnsb,
            tc.tile_pool(name="stagep", bufs=1) as stagep,
            tc.tile_pool(name="outtokp", bufs=2) as outtokp,
            tc.tile_pool(name="outfp", bufs=3) as outfp,
            tc.tile_pool(name="psbig", bufs=4, space="PSUM") as psbig,
            tc.tile_pool(name="psattn", bufs=4, space="PSUM") as psattn,
        ):
            # ---- resident weights / constants ----
            w_sb = pp.tile([128, 8, F3], f32r)
            nc.sync.dma_start(w_sb, wqkv_d.rearrange("(ko kp) f -> kp ko f", kp=128))
            wout_sb = pp.tile([128, 8, E], bf16)
            nc.sync.dma_start(wout_sb, wout_d.rearrange("(ko kp) f -> kp ko f", kp=128))
            bqkv_sb = pp.tile([128, F3 // 128], f32)
            nc.sync.dma_start(bqkv_sb, bqkv_d[:])
            bout_sb = pp.tile([1, E], f32r)
            nc.sync.dma_start(bout_sb, bout_d[:])
            mask2_sb = pp.tile([128, 256], f32)
            nc.sync.dma_start(mask2_sb, mask2_c[:])
            idb_sb = pp.tile([128, 128], bf16)
            nc.sync.dma_start(idb_sb, identb_c[:])
            idf_sb = pp.tile([128, 128], f32r)
            nc.sync.dma_start(idf_sb, identf_c[:].bitcast(f32r))
            ones_sb = pp.tile([1, 128], f32r)
            nc.sync.dma_start(ones_sb, ones_c[:].bitcast(f32r))

            # persistent packs: q h-major [d, head, t] (moving operand),
            # k/v t-major [d, t, head] (stationary needs one free dim)
            qpack = pp.tile([64, H, T], bf16)
            kpack = pp.tile([64, T, H], bf16)
            vpack = pp.tile([65, T, H], bf16)
            nc.gpsimd.memset(vpack[64:65, :, :], 1.0)

            for it in range(niter):
                t0 = it * T
                # ---- load x and transpose on PE -> xT [e, t] ----
                x_sb = xtp.tile([128, T // 128, E], f32r, tag="x_sb")
                for jm in range(T // 128):
                    nc.sync.dma_start(x_sb[:, jm, :], x_d[ds(t0 + jm * 128, 128), :])
                xt_sb = xtp.tile([128, 8, T], f32r)
                for e in range(8):
                    for jm in range(T // 128):
                        pst = psattn.tile([128, 128], f32r, tag="ps_attn")
                        nc.tensor.transpose(
                            pst, x_sb[:, jm, ds(e * 128, 128)], idf_sb[:]
                        )
                        nc.scalar.copy(
                            out=xt_sb[:, e, ds(jm * 128, 128)], in_=pst[:]
                        )

                # ---- QKV projection: qkvT tiles [128f x T] ----
                qkvt = []
                for j in range(F3 // 128):
                    psq_full = psbig.tile([128, 512], f32, tag="ps_big")
                    psq = psq_full[:, :T]
                    for e in range(8):
                        nc.tensor.matmul(
                            psq,
                            w_sb[:, e, ds(j * 128, 128)],
                            xt_sb[:, e, :],
                            start=(e == 0),
                            stop=(e == 7),
                        )
                    qt = qkvtp.tile([128, T], bf16, tag="qkvt")
                    nc.scalar.activation(
                        qt[:],
                        psq,
                        mybir.ActivationFunctionType.Identity,
                        bias=bqkv_sb[:, j : j + 1],
                        scale=1.0,
                    )
                    qkvt.append(qt)

                def slab(frow):
                    jt, off = divmod(frow, 128)
                    return qkvt[jt][off : off + 64, :]

                for h in range(H):
                    nc.vector.tensor_copy(out=qpack[:, h, :], in_=slab(192 * h))
                    nc.vector.tensor_copy(
                        out=kpack[:, :, h], in_=slab(192 * h + 64)
                    )
                    nc.gpsimd.tensor_copy(
                        out=vpack[:64, :, h], in_=slab(192 * h + 128)
                    )

                # ---- attention: 8-token groups, processed in pairs ----
                staging = stagep.tile([64, T, H], bf16, tag="staging")
                for gp in range(NG // 2):
                    gA, gB = 2 * gp, 2 * gp + 1
                    # scoresT for the pair -> one psum tile
                    psS2 = psattn.tile([128, 2, 128], f32, tag="ps_attn")
                    for i, g in enumerate((gA, gB)):
                        nc.tensor.matmul(
                            psS2[:, i, :],
                            kpack[:, ds(g * 8, 8), :].rearrange(
                                "p a b -> p (a b)"
                            ),
                            qpack[:, :, ds(g * 8, 8)],
                            start=True,
                            stop=True,
                        )
                    smask2 = attnsb.tile([128, 256], f32, tag="smask")
                    nc.vector.tensor_add(
                        out=smask2[:],
                        in0=psS2.rearrange("p a b -> p (a b)"),
                        in1=mask2_sb[:],
                    )
                    expS2 = attnsb.tile([128, 256], bf16, tag="expS")
                    nc.scalar.activation(
                        expS2[:],
                        smask2[:],
                        mybir.ActivationFunctionType.Exp,
                        bias=0.0,
                        scale=0.125,
                    )
                    # vT for the pair: [(g,t) rows] x [d|1 cols]
                    psV2 = psattn.tile([128, 2, 66], bf16, tag="ps_attn")
                    for i, g in enumerate((gA, gB)):
                        nc.tensor.transpose(
                            psV2[:, i, :65],
                            vpack[:, ds(g * 8, 8), :].rearrange(
                                "p a b -> p (a b)"
                            ),
                            idb_sb[:65, :65],
                        )
                    vt2_sb = attnsb.tile([128, 2, 65], bf16, tag="vt")
                    nc.scalar.activation(
                        vt2_sb[:],
                        psV2[:, :, :65],
                        mybir.ActivationFunctionType.Copy,
                    )
                    # AV (+ denominator in col 64)
                    psAV2 = psattn.tile([128, 2, 65], f32, tag="ps_attn")
                    for i in range(2):
                        nc.tensor.matmul(
                            psAV2[:, i, :],
                            expS2[:, ds(i * 128, 128)],
                            vt2_sb[:, i, :],
                            start=True,
                            stop=True,
                        )
                    rec2 = attnsb.tile([128, 2], f32, tag="rec")
                    nc.vector.reciprocal(rec2[:], psAV2[:, :, 64])
                    onorm2 = attnsb.tile([128, 2, 64], bf16, tag="onorm")
                    nc.vector.tensor_tensor(
                        onorm2[:],
                        psAV2[:, :, 0:64],
                        rec2[:, :, None].to_broadcast((128, 2, 64)),
                        mybir.AluOpType.mult,
                    )
                    # transpose [(h,t), d-pair] -> [d-pair, (h,t)]
                    psN2 = psattn.tile([128, 128], bf16, tag="ps_attn")
                    nc.tensor.transpose(
                        psN2,
                        onorm2.rearrange("p a b -> p (a b)"),
                        idb_sb[:],
                    )
                    for i, g in enumerate((gA, gB)):
                        nc.vector.tensor_copy(
                            out=staging[:, ds(g * 8, 8), :],
                            in_=psN2[i * 64 : i * 64 + 64, :]
                            .rearrange("p (a b) -> p a b", a=H)
                            .rearrange("p a b -> p b a"),
                        )

                # ---- regroup to outtok [(h*64+d) chunks, t] ----
                outtok = outtokp.tile([128, 8, T], bf16, tag="outtok")
                for h in range(H):
                    nc.gpsimd.tensor_copy(
                        out=outtok[(h % 2) * 64 : (h % 2) * 64 + 64, h // 2, :],
                        in_=staging[:, :, h],
                    )

                # ---- output projection + bias ----
                for jm in range(T // 128):
                    for nh in range(2):
                        psO = psbig.tile([128, 512], f32, tag="ps_big")
                        for k2 in range(8):
                            nc.tensor.matmul(
                                psO,
                                outtok[:, k2, ds(jm * 128, 128)],
                                wout_sb[:, k2, ds(nh * 512, 512)],
                                start=(k2 == 0),
                                stop=False,
                            )
                        nc.tensor.matmul(
                            psO,
                            ones_sb[:, :],
                            bout_sb[:, ds(nh * 512, 512)],
                            start=False,
                            stop=True,
                        )
                        outf = outfp.tile([128, 512], f32, tag="outf")
                        nc.scalar.activation(
                            outf[:], psO, mybir.ActivationFunctionType.Copy
                        )
                        nc.sync.dma_start(
                            out_d[ds(t0 + jm * 128, 128), ds(nh * 512, 512)],
                            outf[:],
                        )
    nc.finalize()
    return nc


_cache = {}


def _get_nc(toks_per_core=TOKS):
    if toks_per_core not in _cache:
        _cache[toks_per_core] = build(toks_per_core)
    return _cache[toks_per_core]


def prep_inputs(x, w_qkv, b_qkv, w_out, b_out, toks_per_core=TOKS, n_cores=N_CORES):
    """Shard tokens over cores; replicate (host-preprocessed) weights."""
    xf = np.ascontiguousarray(x, dtype=np.float32).reshape(-1, E)
    wq = np.ascontiguousarray(w_qkv, dtype=np.float32)
    bq = np.ascontiguousarray(
        np.asarray(b_qkv, dtype=np.float32).reshape(F3 // 128, 128).T
    )
    wo = np.ascontiguousarray(np.asarray(w_out).astype(ml_dtypes.bfloat16))
    bo = np.ascontiguousarray(np.asarray(b_out, dtype=np.float32).reshape(1, E))
    in_maps = []
    for c in range(n_cores):
        in_maps.append(
            {
                "x": np.ascontiguousarray(
                    xf[c * toks_per_core : (c + 1) * toks_per_core]
                ),
                "w_qkv": wq,
                "b_qkv": bq,
                "w_out": wo,
                "b_out": bo,
            }
        )
    return in_maps


def run(x, w_qkv, b_qkv, w_out, b_out, toks_per_core=TOKS, n_cores=N_CORES, **kw):
    from concourse import bass_utils

    nc = _get_nc(toks_per_core)
    in_maps = prep_inputs(
        x, w_qkv, b_qkv, w_out, b_out, toks_per_core, n_cores
    )
    res = bass_utils.run_bass_kernel_spmd(
        nc, in_maps, core_ids=list(range(n_cores)), **kw
    )
    out = np.concatenate([r["out"] for r in res.results], axis=0)
    return out, res


def kernel(x, w_qkv, b_qkv, w_out, b_out):
    out, _ = run(x, w_qkv, b_qkv, w_out, b_out)
    return out.reshape(x.shape[0], x.shape[1], E)


# revision 15
# speedup vs baseline: 2.7142x; 1.3151x over previous
"""Bass/Tile TRN2 kernel for per-token multi-head attention over heads.

Reference computation (per token t):
  qkv = x @ w_qkv + b_qkv                  # [t, 3072]
  q/k/v[h, d] = qkv[h*192 + {0,64,128} + d]
  scores[h, g] = q[h] . k[g] / 8
  attn = softmax(scores, axis=g)
  out[h, d] = sum_g attn[h, g] v[g, d]
  y = out.reshape(1024) @ w_out + b_out

Sharding: tokens (B*S = 32768) split evenly over 8 cores; weights replicated.

Layout notes (v2):
  - qkv computed transposed ([f x t]) so per-head 64-row slabs are clean
    partition ranges; f32r matmuls (full PE rate at N>=256, ~1e-4 rounding).
  - per-head q/k/v repacked h-major into [d, head, t] bf16 tiles with
    contiguous copies; block-diag 128x128 matmuls give 8 tokens' 16x16
    head-attention at once (2 groups share each psum tile). Mask selects
    t==t' pairs; exp is unnormalized, softmax denominator comes from a
    ones-column in the AV matmul.
  - x transposed via DMA-transpose (2 x 64-partition transfers per e-chunk).
"""

import numpy as np
import ml_dtypes

H, DH = 16, 64
E = 1024
F3 = 3072
B, S = 4, 8192
N_CORES = 8
TOKS = (B * S) // N_CORES  # 4096 tokens per core
T = 256                    # tokens per unrolled iteration
NG = T // 8                # 8-token groups per iteration

NEG = -1.0e9


def _consts():
    # scoresT rows a=(t, g) t-major, cols b=(h, t') h-major; valid iff t==t'
    a = np.arange(128)
    mask = np.where((a[:, None] // 16) == (a[None, :] % 8), 0.0, NEG).astype(
        np.float32
    )
    mask2 = np.concatenate([mask, mask], axis=1)  # [128, 256] for group pairs
    ident = np.eye(128, dtype=np.float32)
    return mask2, ident


def build(toks_per_core=TOKS):
    from concourse.bacc import Bacc
    import concourse.mybir as mybir
    from concourse.tile import TileContext
    from concourse.bass import ds

    f32 = mybir.dt.float32
    f32r = mybir.dt.float32r
    bf16 = mybir.dt.bfloat16
    niter = toks_per_core // T

    nc = Bacc("TRN2")
    x_d = nc.dram_tensor("x", [toks_per_core, E], f32r, kind="ExternalInput")
    wqkv_d = nc.dram_tensor("w_qkv", [E, F3], f32r, kind="ExternalInput")
    bqkv_d = nc.dram_tensor("b_qkv", [128, F3 // 128], f32, kind="ExternalInput")
    wout_d = nc.dram_tensor("w_out", [E, E], bf16, kind="ExternalInput")
    bout_d = nc.dram_tensor("b_out", [1, E], bf16, kind="ExternalInput")
    out_d = nc.dram_tensor("out", [toks_per_core, E], f32, kind="ExternalOutput")

    mask2_np, ident_np = _consts()
    mask4_np = np.concatenate([mask2_np, mask2_np], axis=1)  # [128, 512]
    mask4_c = nc.inline_tensor(mask4_np, name="mask4_c")
    identf_c = nc.inline_tensor(ident_np, name="identf_c")
    identb_c = nc.inline_tensor(ident_np.astype(ml_dtypes.bfloat16), name="identb_c")
    ones_c = nc.inline_tensor(np.ones((1, 128), ml_dtypes.bfloat16), name="ones_c")

    # parity-major head slot: even heads 0-7, odd heads 8-15
    def hslot(h):
        return (h % 2) * 8 + h // 2

    with TileContext(nc) as tc:
        with (
            tc.tile_pool(name="persist", bufs=1) as pp,
            tc.tile_pool(name="xtp", bufs=2) as xtp,
            tc.tile_pool(name="attnsb", bufs=4) as attnsb,
            tc.tile_pool(name="outtokp", bufs=2) as outtokp,
            tc.tile_pool(name="outfp", bufs=3) as outfp,
            tc.tile_pool(name="psbig", bufs=4, space="PSUM") as psbig,
            tc.tile_pool(name="psattn", bufs=4, space="PSUM") as psattn,
        ):
            # ---- resident weights / constants ----
            w_sb = pp.tile([128, 8, F3], f32r)
            nc.sync.dma_start(w_sb, wqkv_d.rearrange("(ko kp) f -> kp ko f", kp=128))
            wout_sb = pp.tile([128, 8, E], bf16)
            nc.sync.dma_start(wout_sb, wout_d.rearrange("(ko kp) f -> kp ko f", kp=128))
            bqkv_sb = pp.tile([128, F3 // 128], f32)
            nc.sync.dma_start(bqkv_sb, bqkv_d[:])
            bout_sb = pp.tile([1, E], bf16)
            nc.sync.dma_start(bout_sb, bout_d[:])
            mask4_sb = pp.tile([128, 512], f32)
            nc.sync.dma_start(mask4_sb, mask4_c[:])
            idb_sb = pp.tile([128, 128], bf16)
            nc.sync.dma_start(idb_sb, identb_c[:])
            idf_sb = pp.tile([128, 128], f32r)
            nc.sync.dma_start(idf_sb, identf_c[:].bitcast(f32r))
            ones_sb = pp.tile([1, 128], bf16)
            nc.sync.dma_start(ones_sb, ones_c[:])

            # packs: q h-major [d, slot, t] (moving operand),
            # k/v t-major [d, t, slot] (stationary needs one free dim)
            qpack = pp.tile([64, H, T], bf16)
            kpack = pp.tile([64, T, H], bf16)
            vpack = pp.tile([65, T, H], bf16)
            nc.gpsimd.memset(vpack[64:65, :, :], 1.0)

            # slab(frow) -> (tile index, partition offset) inside qkvT
            for it in range(niter):
                t0 = it * T
                # ---- load x and transpose on PE -> xT [e, t] ----
                x_sb = xtp.tile([128, T // 128, E], f32r, tag="x_sb")
                for jm in range(T // 128):
                    nc.sync.dma_start(x_sb[:, jm, :], x_d[ds(t0 + jm * 128, 128), :])
                xt_sb = xtp.tile([128, 8, T], f32r)
                for e in range(8):
                    for jm in range(T // 128):
                        pst = psattn.tile([128, 128], f32r, tag="ps_attn")
                        nc.tensor.transpose(
                            pst, x_sb[:, jm, ds(e * 128, 128)], idf_sb[:]
                        )
                        nc.scalar.copy(
                            out=xt_sb[:, e, ds(jm * 128, 128)], in_=pst[:]
                        )

                # ---- QKV projection; psum halves drain straight into packs
                # f-tile j covers rows 128j..128j+127 = two 64-row slabs.
                # slab (h, which): frow = 192h + 64*which; which 0/1/2 = q/k/v.
                drain_alt = 0
                for j in range(F3 // 128):
                    psq_full = psbig.tile([128, 512], f32, tag="ps_big")
                    psq = psq_full[:, :T]
                    for e in range(8):
                        nc.tensor.matmul(
                            psq,
                            w_sb[:, e, ds(j * 128, 128)],
                            xt_sb[:, e, :],
                            start=(e == 0),
                            stop=(e == 7),
                        )
                    for half in range(2):
                        frow = j * 128 + half * 64
                        h, rem = divmod(frow, 192)
                        which = rem // 64
                        sl = hslot(h)
                        if which == 0:
                            dst = qpack[:, sl, :]
                        elif which == 1:
                            dst = kpack[:, :, sl]
                        else:
                            dst = vpack[:64, :, sl]
                        src = psq[half * 64 : half * 64 + 64, :]
                        bias_ap = bqkv_sb[half * 64 : half * 64 + 64, j : j + 1]
                        if which == 0 or drain_alt % 2 == 0:
                            nc.scalar.activation(
                                dst,
                                src,
                                mybir.ActivationFunctionType.Identity,
                                bias=bias_ap,
                                scale=1.0,
                            )
                        else:
                            nc.vector.scalar_tensor_tensor(
                                out=dst,
                                in0=src,
                                scalar=1.0,
                                in1=bias_ap.to_broadcast((64, T)),
                                op0=mybir.AluOpType.mult,
                                op1=mybir.AluOpType.add,
                            )
                        if which != 0:
                            drain_alt += 1

                # ---- attention: 8-token groups, 4 per "quad" ----
                outtok = outtokp.tile([128, 8, T], bf16, tag="outtok")
                for q4 in range(NG // 4):
                    gs4 = [4 * q4 + i for i in range(4)]
                    psS4 = psattn.tile([128, 4, 128], f32, tag="ps_attn")
                    for i, g in enumerate(gs4):
                        nc.tensor.matmul(
                            psS4[:, i, :],
                            kpack[:, ds(g * 8, 8), :].rearrange(
                                "p a b -> p (a b)"
                            ),
                            qpack[:, :, ds(g * 8, 8)],
                            start=True,
                            stop=True,
                        )
                    smask4 = attnsb.tile([128, 512], f32, tag="smask")
                    nc.vector.tensor_add(
                        out=smask4[:],
                        in0=psS4.rearrange("p a b -> p (a b)"),
                        in1=mask4_sb[:],
                    )
                    expS4 = attnsb.tile([128, 512], bf16, tag="expS")
                    nc.scalar.activation(
                        expS4[:],
                        smask4[:],
                        mybir.ActivationFunctionType.Exp,
                        bias=0.0,
                        scale=0.125,
                    )
                    psV4 = psattn.tile([128, 4, 66], bf16, tag="ps_attn")
                    for i, g in enumerate(gs4):
                        nc.tensor.transpose(
                            psV4[:, i, :65],
                            vpack[:, ds(g * 8, 8), :].rearrange(
                                "p a b -> p (a b)"
                            ),
                            idb_sb[:65, :65],
                        )
                    vt4_sb = attnsb.tile([128, 4, 65], bf16, tag="vt")
                    nc.scalar.activation(
                        vt4_sb[:],
                        psV4[:, :, :65],
                        mybir.ActivationFunctionType.Copy,
                    )
                    psAV4 = psattn.tile([128, 4, 65], f32, tag="ps_attn")
                    for i in range(4):
                        nc.tensor.matmul(
                            psAV4[:, i, :],
                            expS4[:, ds(i * 128, 128)],
                            vt4_sb[:, i, :],
                            start=True,
                            stop=True,
                        )
                    rec4 = attnsb.tile([128, 4], f32, tag="rec")
                    nc.vector.reciprocal(rec4[:], psAV4[:, :, 64])
                    onorm4 = attnsb.tile([128, 4, 64], bf16, tag="onorm")
                    nc.vector.tensor_tensor(
                        onorm4[:],
                        psAV4[:, :, 0:64],
                        rec4[:, :, None].to_broadcast((128, 4, 64)),
                        mybir.AluOpType.mult,
                    )
                    # two pair-transposes into one psum tile
                    psN4 = psattn.tile([128, 2, 128], bf16, tag="ps_attn")
                    for p in range(2):
                        nc.tensor.transpose(
                            psN4[:, p, :],
                            onorm4[:, 2 * p : 2 * p + 2, :].rearrange(
                                "p a b -> p (a b)"
                            ),
                            idb_sb[:],
                        )
                    # regroup into outtok: per (pair, group-half, parity)
                    cp_alt = 0
                    for p in range(2):
                        for i in range(2):
                            g = gs4[2 * p + i]
                            for par in range(2):
                                src = psN4[
                                    i * 64 : i * 64 + 64, p, ds(par * 64, 64)
                                ].rearrange("p (a b) -> p a b", a=8)
                                dst = outtok[
                                    par * 64 : par * 64 + 64, :, ds(g * 8, 8)
                                ]
                                if cp_alt % 2 == 0:
                                    nc.vector.tensor_copy(out=dst, in_=src)
                                else:
                                    nc.scalar.copy(out=dst, in_=src)
                                cp_alt += 1

                # ---- output projection + bias ----
                for jm in range(T // 128):
                    for nh in range(2):
                        psO = psbig.tile([128, 512], f32, tag="ps_big")
                        for k2 in range(8):
                            nc.tensor.matmul(
                                psO,
                                outtok[:, k2, ds(jm * 128, 128)],
                                wout_sb[:, k2, ds(nh * 512, 512)],
                                start=(k2 == 0),
                                stop=False,
                            )
                        nc.tensor.matmul(
                            psO,
                            ones_sb[:, :],
                            bout_sb[:, ds(nh * 512, 512)],
                            start=False,
                            stop=True,
                        )
                        outf = outfp.tile([128, 512], f32, tag="outf")
                        nc.scalar.activation(
                            outf[:], psO, mybir.ActivationFunctionType.Copy
                        )
                        nc.sync.dma_start(
                            out_d[ds(t0 + jm * 128, 128), ds(nh * 512, 512)],
                            outf[:],
                        )
    nc.finalize()
    return nc


_cache = {}


def _get_nc(toks_per_core=TOKS):
    if toks_per_core not in _cache:
        _cache[toks_per_core] = build(toks_per_core)
    return _cache[toks_per_core]


def prep_inputs(x, w_qkv, b_qkv, w_out, b_out, toks_per_core=TOKS, n_cores=N_CORES):
    """Shard tokens over cores; replicate (host-preprocessed) weights."""
    xf = np.ascontiguousarray(x, dtype=np.float32).reshape(-1, E)
    wq = np.ascontiguousarray(w_qkv, dtype=np.float32)
    bq = np.ascontiguousarray(
        np.asarray(b_qkv, dtype=np.float32).reshape(F3 // 128, 128).T
    )
    wo = np.ascontiguousarray(np.asarray(w_out).astype(ml_dtypes.bfloat16))
    bo = np.ascontiguousarray(np.asarray(b_out, dtype=np.float32).astype(ml_dtypes.bfloat16).reshape(1, E))
    in_maps = []
    for c in range(n_cores):
        in_maps.append(
            {
                "x": np.ascontiguousarray(
                    xf[c * toks_per_core : (c + 1) * toks_per_core]
                ),
                "w_qkv": wq,
                "b_qkv": bq,
                "w_out": wo,
                "b_out": bo,
            }
        )
    return in_maps


def run(x, w_qkv, b_qkv, w_out, b_out, toks_per_core=TOKS, n_cores=N_CORES, **kw):
    from concourse import bass_utils

    nc = _get_nc(toks_per_core)
    in_maps = prep_inputs(
        x, w_qkv, b_qkv, w_out, b_out, toks_per_core, n_cores
    )
    res = bass_utils.run_bass_kernel_spmd(
        nc, in_maps, core_ids=list(range(n_cores)), **kw
    )
    out = np.concatenate([r["out"] for r in res.results], axis=0)
    return out, res


def kernel(x, w_qkv, b_qkv, w_out, b_out):
    out, _ = run(x, w_qkv, b_qkv, w_out, b_out)
    return out.reshape(x.shape[0], x.shape[1], E)


# revision 16
# speedup vs baseline: 3.2734x; 1.2061x over previous
"""Bass/Tile TRN2 kernel for per-token multi-head attention over heads.

Reference computation (per token t):
  qkv = x @ w_qkv + b_qkv                  # [t, 3072]
  q/k/v[h, d] = qkv[h*192 + {0,64,128} + d]
  scores[h, g] = q[h] . k[g] / 8
  attn = softmax(scores, axis=g)
  out[h, d] = sum_g attn[h, g] v[g, d]
  y = out.reshape(1024) @ w_out + b_out

Sharding: tokens (B*S = 32768) split evenly over 8 cores; weights replicated.

Layout notes (v2):
  - qkv computed transposed ([f x t]) so per-head 64-row slabs are clean
    partition ranges; f32r matmuls (full PE rate at N>=256, ~1e-4 rounding).
  - per-head q/k/v repacked h-major into [d, head, t] bf16 tiles with
    contiguous copies; block-diag 128x128 matmuls give 8 tokens' 16x16
    head-attention at once (2 groups share each psum tile). Mask selects
    t==t' pairs; exp is unnormalized, softmax denominator comes from a
    ones-column in the AV matmul.
  - x transposed via DMA-transpose (2 x 64-partition transfers per e-chunk).
"""

import numpy as np
import ml_dtypes

H, DH = 16, 64
E = 1024
F3 = 3072
B, S = 4, 8192
N_CORES = 8
TOKS = (B * S) // N_CORES  # 4096 tokens per core
T = 256                    # tokens per unrolled iteration
NG = T // 8                # 8-token groups per iteration

NEG = -1.0e9


def _consts():
    # scoresT rows a=(slot_k, t), cols b=(slot_q, t'); valid iff t==t'
    a = np.arange(128)
    mask = np.where((a[:, None] % 8) == (a[None, :] % 8), 0.0, NEG).astype(
        np.float32
    )
    mask2 = np.concatenate([mask, mask], axis=1)  # [128, 256] for group pairs
    ident = np.eye(128, dtype=np.float32)
    return mask2, ident


def build(toks_per_core=TOKS):
    from concourse.bacc import Bacc
    import concourse.mybir as mybir
    from concourse.tile import TileContext
    from concourse.bass import ds

    f32 = mybir.dt.float32
    f32r = mybir.dt.float32r
    bf16 = mybir.dt.bfloat16
    niter = toks_per_core // T

    nc = Bacc("TRN2")
    x_d = nc.dram_tensor("x", [toks_per_core, E], f32r, kind="ExternalInput")
    wqkv_d = nc.dram_tensor("w_qkv", [E, F3], f32r, kind="ExternalInput")
    bqkv_d = nc.dram_tensor("b_qkv", [128, F3 // 128], f32, kind="ExternalInput")
    wout_d = nc.dram_tensor("w_out", [E, E], bf16, kind="ExternalInput")
    bout_d = nc.dram_tensor("b_out", [1, E], bf16, kind="ExternalInput")
    out_d = nc.dram_tensor("out", [toks_per_core, E], f32, kind="ExternalOutput")

    mask2_np, ident_np = _consts()
    mask4_np = np.concatenate([mask2_np, mask2_np], axis=1)  # [128, 512]
    mask4_c = nc.inline_tensor(mask4_np, name="mask4_c")
    identf_c = nc.inline_tensor(ident_np, name="identf_c")
    identb_c = nc.inline_tensor(ident_np.astype(ml_dtypes.bfloat16), name="identb_c")
    ones_c = nc.inline_tensor(np.ones((1, 128), ml_dtypes.bfloat16), name="ones_c")

    # parity-major head slot: even heads 0-7, odd heads 8-15
    def hslot(h):
        return (h % 2) * 8 + h // 2

    with TileContext(nc) as tc:
        with (
            tc.tile_pool(name="persist", bufs=1) as pp,
            tc.tile_pool(name="xtp", bufs=2) as xtp,
            tc.tile_pool(name="attnsb", bufs=4) as attnsb,
            tc.tile_pool(name="outtokp", bufs=2) as outtokp,
            tc.tile_pool(name="outfp", bufs=3) as outfp,
            tc.tile_pool(name="psbig", bufs=4, space="PSUM") as psbig,
            tc.tile_pool(name="psattn", bufs=4, space="PSUM") as psattn,
        ):
            # ---- resident weights / constants ----
            w_sb = pp.tile([128, 8, F3], f32r)
            nc.sync.dma_start(w_sb, wqkv_d.rearrange("(ko kp) f -> kp ko f", kp=128))
            wout_sb = pp.tile([128, 8, E], bf16)
            nc.sync.dma_start(wout_sb, wout_d.rearrange("(ko kp) f -> kp ko f", kp=128))
            bqkv_sb = pp.tile([128, F3 // 128], f32)
            nc.sync.dma_start(bqkv_sb, bqkv_d[:])
            bout_sb = pp.tile([1, E], bf16)
            nc.sync.dma_start(bout_sb, bout_d[:])
            mask4_sb = pp.tile([128, 512], f32)
            nc.sync.dma_start(mask4_sb, mask4_c[:])
            idb_sb = pp.tile([128, 128], bf16)
            nc.sync.dma_start(idb_sb, identb_c[:])
            idf_sb = pp.tile([128, 128], f32r)
            nc.sync.dma_start(idf_sb, identf_c[:].bitcast(f32r))
            ones_sb = pp.tile([1, 128], bf16)
            nc.sync.dma_start(ones_sb, ones_c[:])

            # packs: q slot-major [d, slot, t] (moving operand);
            # k/v [d, t_hi, slot, t_lo] so group slab [:, g] is contiguous
            # (stationary needs one free dim) and drains write 8-elem runs
            qpack = pp.tile([64, H, T], bf16)
            kpack = pp.tile([64, T // 8, H, 8], bf16)
            vpack = pp.tile([65, T // 8, H, 8], bf16)
            nc.gpsimd.memset(vpack[64:65, :, :, :], 1.0)

            # slab(frow) -> (tile index, partition offset) inside qkvT
            for it in range(niter):
                t0 = it * T
                # ---- load x and transpose on PE -> xT [e, t] ----
                x_sb = xtp.tile([128, T // 128, E], f32r, tag="x_sb")
                for jm in range(T // 128):
                    nc.sync.dma_start(x_sb[:, jm, :], x_d[ds(t0 + jm * 128, 128), :])
                xt_sb = xtp.tile([128, 8, T], f32r)
                for e in range(8):
                    for jm in range(T // 128):
                        pst = psattn.tile([128, 128], f32r, tag="ps_attn")
                        nc.tensor.transpose(
                            pst, x_sb[:, jm, ds(e * 128, 128)], idf_sb[:]
                        )
                        if (e + jm) % 2 == 0:
                            nc.scalar.copy(
                                out=xt_sb[:, e, ds(jm * 128, 128)], in_=pst[:]
                            )
                        else:
                            nc.vector.tensor_copy(
                                out=xt_sb[:, e, ds(jm * 128, 128)], in_=pst[:]
                            )

                # ---- QKV projection; psum halves drain straight into packs
                # f-tile j covers rows 128j..128j+127 = two 64-row slabs.
                # slab (h, which): frow = 192h + 64*which; which 0/1/2 = q/k/v.
                drain_alt = 0
                for j in range(F3 // 128):
                    psq_full = psbig.tile([128, 512], f32, tag="ps_big")
                    psq = psq_full[:, :T]
                    for e in range(8):
                        nc.tensor.matmul(
                            psq,
                            w_sb[:, e, ds(j * 128, 128)],
                            xt_sb[:, e, :],
                            start=(e == 0),
                            stop=(e == 7),
                        )
                    for half in range(2):
                        frow = j * 128 + half * 64
                        h, rem = divmod(frow, 192)
                        which = rem // 64
                        sl = hslot(h)
                        if which == 0:
                            dst = qpack[:, sl, :]
                        elif which == 1:
                            dst = kpack[:, :, sl, :]
                        else:
                            dst = vpack[:64, :, sl, :]
                        src = psq[half * 64 : half * 64 + 64, :]
                        bias_ap = bqkv_sb[half * 64 : half * 64 + 64, j : j + 1]
                        if which == 0 or drain_alt % 2 == 0:
                            nc.scalar.activation(
                                dst,
                                src,
                                mybir.ActivationFunctionType.Identity,
                                bias=bias_ap,
                                scale=1.0,
                            )
                        else:
                            nc.vector.scalar_tensor_tensor(
                                out=dst,
                                in0=src.rearrange("p (a b) -> p a b", b=8),
                                scalar=1.0,
                                in1=bias_ap[:, :, None].to_broadcast(
                                    (64, T // 8, 8)
                                ),
                                op0=mybir.AluOpType.mult,
                                op1=mybir.AluOpType.add,
                            )
                        if which != 0:
                            drain_alt += 1

                # ---- attention: 8-token groups, 4 per "quad" ----
                outtok = outtokp.tile([128, 8, T], bf16, tag="outtok")
                for q4 in range(NG // 4):
                    gs4 = [4 * q4 + i for i in range(4)]
                    psS4 = psattn.tile([128, 4, 128], f32, tag="ps_attn")
                    for i, g in enumerate(gs4):
                        nc.tensor.matmul(
                            psS4[:, i, :],
                            kpack[:, g, :, :].rearrange("p a b -> p (a b)"),
                            qpack[:, :, ds(g * 8, 8)],
                            start=True,
                            stop=True,
                        )
                    smask4 = attnsb.tile([128, 512], f32, tag="smask")
                    nc.vector.tensor_add(
                        out=smask4[:],
                        in0=psS4.rearrange("p a b -> p (a b)"),
                        in1=mask4_sb[:],
                    )
                    expS4 = attnsb.tile([128, 512], bf16, tag="expS")
                    nc.scalar.activation(
                        expS4[:],
                        smask4[:],
                        mybir.ActivationFunctionType.Exp,
                        bias=0.0,
                        scale=0.125,
                    )
                    psV4 = psattn.tile([128, 4, 66], bf16, tag="ps_attn")
                    for i, g in enumerate(gs4):
                        nc.tensor.transpose(
                            psV4[:, i, :65],
                            vpack[:, g, :, :].rearrange("p a b -> p (a b)"),
                            idb_sb[:65, :65],
                        )
                    vt4_sb = attnsb.tile([128, 4, 65], bf16, tag="vt")
                    nc.scalar.activation(
                        vt4_sb[:],
                        psV4[:, :, :65],
                        mybir.ActivationFunctionType.Copy,
                    )
                    psAV4 = psattn.tile([128, 4, 65], f32, tag="ps_attn")
                    for i in range(4):
                        nc.tensor.matmul(
                            psAV4[:, i, :],
                            expS4[:, ds(i * 128, 128)],
                            vt4_sb[:, i, :],
                            start=True,
                            stop=True,
                        )
                    rec4 = attnsb.tile([128, 4], f32, tag="rec")
                    nc.vector.reciprocal(rec4[:], psAV4[:, :, 64])
                    onorm4 = attnsb.tile([128, 4, 64], bf16, tag="onorm")
                    nc.vector.tensor_tensor(
                        onorm4[:],
                        psAV4[:, :, 0:64],
                        rec4[:, :, None].to_broadcast((128, 4, 64)),
                        mybir.AluOpType.mult,
                    )
                    # two pair-transposes into one psum tile
                    psN4 = psattn.tile([128, 2, 128], bf16, tag="ps_attn")
                    for p in range(2):
                        nc.tensor.transpose(
                            psN4[:, p, :],
                            onorm4[:, 2 * p : 2 * p + 2, :].rearrange(
                                "p a b -> p (a b)"
                            ),
                            idb_sb[:],
                        )
                    # regroup into outtok: per (pair, group-half, parity)
                    cp_alt = 0
                    for p in range(2):
                        for i in range(2):
                            g = gs4[2 * p + i]
                            for par in range(2):
                                src = psN4[
                                    i * 64 : i * 64 + 64, p, ds(par * 64, 64)
                                ].rearrange("p (a b) -> p a b", a=8)
                                dst = outtok[
                                    par * 64 : par * 64 + 64, :, ds(g * 8, 8)
                                ]
                                if cp_alt % 2 == 0:
                                    nc.vector.tensor_copy(out=dst, in_=src)
                                else:
                                    nc.scalar.copy(out=dst, in_=src)
                                cp_alt += 1

                # ---- output projection + bias ----
                for jm in range(T // 128):
                    for nh in range(2):
                        psO = psbig.tile([128, 512], f32, tag="ps_big")
                        for k2 in range(8):
                            nc.tensor.matmul(
                                psO,
                                outtok[:, k2, ds(jm * 128, 128)],
                                wout_sb[:, k2, ds(nh * 512, 512)],
                                start=(k2 == 0),
                                stop=False,
                            )
                        nc.tensor.matmul(
                            psO,
                            ones_sb[:, :],
                            bout_sb[:, ds(nh * 512, 512)],
                            start=False,
                            stop=True,
                        )
                        outf = outfp.tile([128, 512], f32, tag="outf")
                        nc.scalar.activation(
                            outf[:], psO, mybir.ActivationFunctionType.Copy
                        )
                        nc.sync.dma_start(
                            out_d[ds(t0 + jm * 128, 128), ds(nh * 512, 512)],
                            outf[:],
                        )
    nc.finalize()
    return nc


_cache = {}


def _get_nc(toks_per_core=TOKS):
    if toks_per_core not in _cache:
        _cache[toks_per_core] = build(toks_per_core)
    return _cache[toks_per_core]


def prep_inputs(x, w_qkv, b_qkv, w_out, b_out, toks_per_core=TOKS, n_cores=N_CORES):
    """Shard tokens over cores; replicate (host-preprocessed) weights."""
    xf = np.ascontiguousarray(x, dtype=np.float32).reshape(-1, E)
    wq = np.ascontiguousarray(w_qkv, dtype=np.float32)
    bq = np.ascontiguousarray(
        np.asarray(b_qkv, dtype=np.float32).reshape(F3 // 128, 128).T
    )
    wo = np.ascontiguousarray(np.asarray(w_out).astype(ml_dtypes.bfloat16))
    bo = np.ascontiguousarray(np.asarray(b_out, dtype=np.float32).astype(ml_dtypes.bfloat16).reshape(1, E))
    in_maps = []
    for c in range(n_cores):
        in_maps.append(
            {
                "x": np.ascontiguousarray(
                    xf[c * toks_per_core : (c + 1) * toks_per_core]
                ),
                "w_qkv": wq,
                "b_qkv": bq,
                "w_out": wo,
                "b_out": bo,
            }
        )
    return in_maps


def run(x, w_qkv, b_qkv, w_out, b_out, toks_per_core=TOKS, n_cores=N_CORES, **kw):
    from concourse import bass_utils

    nc = _get_nc(toks_per_core)
    in_maps = prep_inputs(
        x, w_qkv, b_qkv, w_out, b_out, toks_per_core, n_cores
    )
    res = bass_utils.run_bass_kernel_spmd(
        nc, in_maps, core_ids=list(range(n_cores)), **kw
    )
    out = np.concatenate([r["out"] for r in res.results], axis=0)
    return out, res


def kernel(x, w_qkv, b_qkv, w_out, b_out):
    out, _ = run(x, w_qkv, b_qkv, w_out, b_out)
    return out.reshape(x.shape[0], x.shape[1], E)
